# revision 3
# baseline (speedup 1.0000x reference)
"""Causal self-attention (single head, d=1024) on 8 Trainium2 NeuronCores.

Problem: x [4, 2048, 1024] f32, Wq/Wk/Wv [1024, 1024] f32
         out[b] = softmax(causal((x@Wq)(x@Wk)^T / 32)) @ (x@Wv)

Sharding: 8 cores = 4 batches x 2 query-shards. Per batch, the 2048 queries
are split into 8 global chunks of 256; core parity p takes global chunks
{2j+p : j=0..3} (interleaved so causal work is balanced). Each core computes
Q projection for its 1024 queries, K/V projection for the full 2048 keys of
its batch (redundantly with its partner - no collectives needed), then
flash-style unnormalized attention:

  QT[e, q]   = sum_d Wq[d, e] xqT[d, q]      (lhsT=Wq tile, rhs=xqT)
  KT[e, k]   = sum_d Wk[d, e] xT[d, k]
  V[k, e]    = sum_d xT[d, k] Wv[d, e]       (lhsT=xT tile,  rhs=Wv)
  S^T[k, q]  = sum_e KT[e, k] QT[e, q]       (no transposes anywhere)
  es         = exp(S^T / 32)  (no max-subtraction: logits ~N(0,1), safe)
  es[diag]  *= mask           (only the first active 256-col group per key
                               block can straddle the causal diagonal)
  attv[q, e] = sum_k es[k, q] V[k, e]        (lhsT=es tile, rhs=V)
  den[q]     = sum_k es[k, q] * 1            (extra N=1 matmul, same lhsT)
  out        = attv * (1/den)

Causal savings: local chunk j only needs key blocks [0, 4j+4) - compiled
extents {4,8,12,16} of 16, i.e. 62.5% of the full score/attv work.
All matmuls in bf16 with fp32 PSUM accumulation.
"""

import copy as _copy
import sys

for _p in ("/opt/trn_rl_repo", "/root/.axon_site/_ro/trn_rl_repo"):
    if _p not in sys.path:
        sys.path.append(_p)

import numpy as np
import ml_dtypes

import concourse.bass as bass
import concourse.mybir as mybir
from concourse.tile import TileContext
from concourse import bass_utils

BF16 = mybir.dt.bfloat16
F32 = mybir.dt.float32

B, T, D = 4, 2048, 1024
NCORES = 8
P = 128
ND = D // P            # 8 contraction tiles over d_in
NE = D // P            # 8 output-feature tiles
NKB = T // P           # 16 key blocks
NCH = 4                # local query chunks per core
CH = 256               # chunk width
DQ = NCH * CH          # 1024 local queries per core
SCALE = 1.0 / np.sqrt(np.float32(D))  # 1/32


def _split_multiwait(nc):
    """This walrus build rejects >1-2 sync waits per instruction for several
    encodings (CTRL drains, PSEUDO_DMA...: "Too many sync wait commands").
    Tile can emit many waits on one instruction. Hoist all but the last wait
    of any multi-wait instruction onto NoOps on the same engine immediately
    before it - same-engine program order makes this equivalent."""
    for f in nc.m.functions:
        for bb in f.blocks:
            newlist = []
            changed = False
            for ins in bb.instructions:
                si = ins.sync_info
                waits = list(si.on_wait) if si and si.on_wait else []
                if len(waits) > 1:
                    changed = True
                    extra, keep = waits[:-1], waits[-1:]
                    for i, w in enumerate(extra):
                        nop = mybir.InstNoOp(
                            name=f"{ins.name}-sw{i}",
                            opcode="NoOp",
                            engine=ins.engine,
                            sync_info=mybir.SyncInfo(on_wait=[w], on_update=[]),
                        )
                        newlist.append(nop)
                    ins.sync_info = mybir.SyncInfo(
                        on_wait=keep, on_update=list(si.on_update)
                    )
                newlist.append(ins)
            if changed:
                bb.instructions = newlist


def _build():
    nc = bass.Bass("TRN2", target_bir_lowering=False, debug=False, num_devices=NCORES)

    xT = nc.declare_dram_parameter("xT", [D, T], BF16, isOutput=False)
    xqT = nc.declare_dram_parameter("xqT", [D, DQ], BF16, isOutput=False)
    wq_d = nc.declare_dram_parameter("Wq", [D, D], BF16, isOutput=False)
    wk_d = nc.declare_dram_parameter("Wk", [D, D], BF16, isOutput=False)
    wv_d = nc.declare_dram_parameter("Wv", [D, D], BF16, isOutput=False)
    msk = nc.declare_dram_parameter("mask", [T, CH], BF16, isOutput=False)
    out = nc.declare_dram_parameter("out", [DQ, D], F32, isOutput=True)

    exp_f = mybir.ActivationFunctionType.Exp

    with TileContext(nc) as tc:
        with (
            tc.tile_pool(name="pqt", bufs=NE) as pqt,
            tc.tile_pool(name="pkt", bufs=NE) as pkt,
            tc.tile_pool(name="pv", bufs=NKB) as pv,
            tc.tile_pool(name="pconst", bufs=1) as pconst,
            tc.tile_pool(name="pmm", bufs=2, space="PSUM") as pmm,
            tc.tile_pool(name="pattv", bufs=4, space="PSUM") as pattv,
            tc.tile_pool(name="pden", bufs=2, space="PSUM") as pden,
        ):
            ones = pconst.tile([P, 8], BF16)
            nc.vector.memset(ones, 1.0)

            qt = []
            kt = []
            vt = []
            with (
                tc.tile_pool(name="px", bufs=ND) as px,
                tc.tile_pool(name="pxq", bufs=ND) as pxq,
                tc.tile_pool(name="pwq", bufs=ND) as pwq,
                tc.tile_pool(name="pwk", bufs=ND) as pwk,
                tc.tile_pool(name="pwv", bufs=ND) as pwv,
            ):
                xts, xqts, wqts, wkts, wvts = [], [], [], [], []
                for d in range(ND):
                    sl = slice(d * P, (d + 1) * P)
                    t = px.tile([P, T], BF16, name=f"xt{d}", tag="xt")
                    nc.sync.dma_start(out=t, in_=xT[sl, :])
                    xts.append(t)
                    t = pxq.tile([P, DQ], BF16, name=f"xqt{d}", tag="xqt")
                    nc.sync.dma_start(out=t, in_=xqT[sl, :])
                    xqts.append(t)
                    t = pwq.tile([P, D], BF16, name=f"wqt{d}", tag="wqt")
                    nc.sync.dma_start(out=t, in_=wq_d[sl, :])
                    wqts.append(t)
                    t = pwk.tile([P, D], BF16, name=f"wkt{d}", tag="wkt")
                    nc.sync.dma_start(out=t, in_=wk_d[sl, :])
                    wkts.append(t)
                    t = pwv.tile([P, D], BF16, name=f"wvt{d}", tag="wvt")
                    nc.sync.dma_start(out=t, in_=wv_d[sl, :])
                    wvts.append(t)

                # Phase 1: QT[e] = [P, DQ]  (transposed queries, bf16)
                for e in range(NE):
                    qte = pqt.tile([P, DQ], BF16, name=f"qt{e}", tag="qt")
                    qt.append(qte)
                    esl = slice(e * P, (e + 1) * P)
                    for g in range(DQ // 512):
                        gs = slice(g * 512, (g + 1) * 512)
                        ps = pmm.tile([P, 512], F32, name="psq", tag="mm")
                        for d in range(ND):
                            nc.tensor.matmul(
                                ps,
                                lhsT=wqts[d][:, esl],
                                rhs=xqts[d][:, gs],
                                start=(d == 0),
                                stop=(d == ND - 1),
                            )
                        nc.scalar.copy(qte[:, gs], ps)

                # Phase 2: KT[e] = [P, T]  (transposed keys, bf16)
                for e in range(NE):
                    kte = pkt.tile([P, T], BF16, name=f"kt{e}", tag="kt")
                    kt.append(kte)
                    esl = slice(e * P, (e + 1) * P)
                    for g in range(T // 512):
                        gs = slice(g * 512, (g + 1) * 512)
                        ps = pmm.tile([P, 512], F32, name="psk", tag="mm")
                        for d in range(ND):
                            nc.tensor.matmul(
                                ps,
                                lhsT=wkts[d][:, esl],
                                rhs=xts[d][:, gs],
                                start=(d == 0),
                                stop=(d == ND - 1),
                            )
                        nc.scalar.copy(kte[:, gs], ps)

                # Phase 3: V[kb] = [P, D]  (natural layout, bf16)
                for kb in range(NKB):
                    vk = pv.tile([P, D], BF16, name=f"v{kb}", tag="v")
                    vt.append(vk)
                    ksl = slice(kb * P, (kb + 1) * P)
                    for g in range(D // 512):
                        gs = slice(g * 512, (g + 1) * 512)
                        ps = pmm.tile([P, 512], F32, name="psv", tag="mm")
                        for d in range(ND):
                            nc.tensor.matmul(
                                ps,
                                lhsT=xts[d][:, ksl],
                                rhs=wvts[d][:, gs],
                                start=(d == 0),
                                stop=(d == ND - 1),
                            )
                        nc.scalar.copy(vk[:, gs], ps)

            # px/pxq/pw* released; reuse that SBUF for scores and output.
            with (
                tc.tile_pool(name="pes", bufs=NCH) as pes,
                tc.tile_pool(name="pmsk", bufs=NKB) as pmsk,
                tc.tile_pool(name="pout", bufs=2) as pout,
                tc.tile_pool(name="psm", bufs=4) as psm,
            ):
                # Phase 4: es[kb] = exp(S^T/32) for local q columns [qlo, DQ)
                es = []
                for kb in range(NKB):
                    qlo = (kb // NCH) * CH
                    wdt = DQ - qlo
                    ksl = slice(kb * P, (kb + 1) * P)
                    t_es = pes.tile([P, wdt], BF16, name=f"es{kb}", tag=f"es{wdt}")
                    es.append((t_es, qlo))
                    mt = pmsk.tile([P, CH], BF16, name=f"mk{kb}", tag="mk")
                    nc.sync.dma_start(out=mt, in_=msk[ksl, :])
                    for g in range(wdt // CH):
                        gs = slice(g * CH, (g + 1) * CH)
                        qs = slice(qlo + g * CH, qlo + (g + 1) * CH)
                        ps = pmm.tile([P, CH], F32, name="pss", tag="mm")
                        for e in range(NE):
                            nc.tensor.matmul(
                                ps,
                                lhsT=kt[e][:, ksl],
                                rhs=qt[e][:, qs],
                                start=(e == 0),
                                stop=(e == NE - 1),
                            )
                        nc.scalar.activation(t_es[:, gs], ps, exp_f, scale=float(SCALE))
                    # only the first group can straddle the causal diagonal
                    nc.vector.tensor_mul(t_es[:, 0:CH], t_es[:, 0:CH], mt)

                # Phase 5: attv + denominator + normalize + store
                for qb in range(DQ // P):
                    ext = 4 * (qb // 2) + 4  # key blocks needed by this q block
                    pa0 = pattv.tile([P, 512], F32, name=f"pa0_{qb}", tag="attv")
                    pa1 = pattv.tile([P, 512], F32, name=f"pa1_{qb}", tag="attv")
                    pd = pden.tile([P, 8], F32, name=f"pd{qb}", tag="den")
                    for kb in range(ext):
                        t_es, qlo = es[kb]
                        lh = t_es[:, qb * P - qlo : qb * P - qlo + P]
                        st = kb == 0
                        sp = kb == ext - 1
                        nc.tensor.matmul(pa0, lhsT=lh, rhs=vt[kb][:, 0:512], start=st, stop=sp)
                        nc.tensor.matmul(pa1, lhsT=lh, rhs=vt[kb][:, 512:1024], start=st, stop=sp)
                        nc.tensor.matmul(pd[:, 0:1], lhsT=lh, rhs=ones[:, 0:1], start=st, stop=sp)
                    rd = psm.tile([P, 1], F32, name=f"rd{qb}", tag="rd")
                    nc.vector.reciprocal(rd, pd[:, 0:1])
                    ot = pout.tile([P, D], F32, name=f"ot{qb}", tag="ot")
                    nc.vector.tensor_scalar_mul(ot[:, 0:512], pa0, rd)
                    nc.vector.tensor_scalar_mul(ot[:, 512:1024], pa1, rd)
                    nc.sync.dma_start(out=out[qb * P : (qb + 1) * P, :], in_=ot)

    _split_multiwait(nc)
    return nc


_NC = None


def _get_nc():
    global _NC
    if _NC is None:
        _NC = _build()
    return _NC


def _local_to_global_q(p):
    """Map local query index [0, DQ) of a parity-p core to global [0, T)."""
    l = np.arange(DQ)
    return CH * (2 * (l // CH) + p) + (l % CH)


def _make_inputs(x, Wq, Wk, Wv):
    bf = ml_dtypes.bfloat16
    wqb = np.ascontiguousarray(Wq.astype(bf))
    wkb = np.ascontiguousarray(Wk.astype(bf))
    wvb = np.ascontiguousarray(Wv.astype(bf))

    # per-parity causal mask for the first active 256-col group of each kb
    masks = {}
    for p in range(2):
        m = np.zeros((T, CH), dtype=bf)
        k = np.arange(T)[:, None]
        for kb in range(NKB):
            j0 = kb // NCH
            g = CH * (2 * j0 + p) + np.arange(CH)[None, :]
            rows = slice(kb * P, (kb + 1) * P)
            m[rows] = (k[rows] <= g).astype(bf)
        masks[p] = m

    in_maps = []
    for c in range(NCORES):
        b, p = c // 2, c % 2
        xTb = np.ascontiguousarray(x[b].T.astype(bf))  # [D, T]
        cols = _local_to_global_q(p)
        xqT = np.ascontiguousarray(xTb[:, cols])
        in_maps.append(
            {"xT": xTb, "xqT": xqT, "Wq": wqb, "Wk": wkb, "Wv": wvb, "mask": masks[p]}
        )
    return in_maps


def _assemble(results, dtype=np.float32):
    y = np.empty((B, T, D), dtype=dtype)
    for c in range(NCORES):
        b, p = c // 2, c % 2
        y[b, _local_to_global_q(p), :] = results[c]["out"]
    return y


def run_spmd(x, Wq, Wk, Wv, **kwargs):
    """Run the kernel; returns (full_output, BassKernelResults)."""
    nc = _get_nc()
    in_maps = _make_inputs(
        np.asarray(x, np.float32),
        np.asarray(Wq, np.float32),
        np.asarray(Wk, np.float32),
        np.asarray(Wv, np.float32),
    )
    r = bass_utils.run_bass_kernel_spmd(nc, in_maps, core_ids=list(range(NCORES)), **kwargs)
    return _assemble(r.results), r


def kernel(x, Wq, Wk, Wv):
    y, _ = run_spmd(x, Wq, Wk, Wv)
    return y
